# revision 34
# baseline (speedup 1.0000x reference)
"""Distributed cross-attention kernel for 8 TRN2 NeuronCores.

Reference computation (L=4096, D_MODEL=1024, D_ATTN=D_MID=128):
    q = x @ Wq + bq ; k = z @ Wk + bk ; v = z @ Wv + bv
    y = softmax(q @ k.T / sqrt(128)) @ v

Sharding: query rows (L_x) split 8 ways; each core holds its x shard and a
replicated copy of z / weights, computes k/v locally, and runs a
flash-attention-style pipeline over 8 z-column tiles of 512.

Matmul inputs are bf16 (accumulation stays fp32 in PSUM; softmax sums and
the normalization run in fp32), which halves HBM traffic and runs the PE
at full rate with pipelined weight loads.  No max-subtraction in the
softmax: s ~ N(0,1) here so exp() is safely bounded.

Host-side work is layout only: transpose/pack/cast the inputs into the
exact SBUF tile layouts (so every DMA is one contiguous read), and
re-stack the 8 output shards.
"""
import math
import sys

import numpy as np

sys.path.insert(0, "/opt/trn_rl_repo")

import ml_dtypes  # noqa: E402

import concourse.mybir as mybir  # noqa: E402
from concourse import bacc  # noqa: E402
from concourse.bass_utils import run_bass_kernel_spmd  # noqa: E402
from concourse.tile import TileContext  # noqa: E402

N_CORES = 8
L = 4096
D_MODEL = 1024
D_ATTN = 128
D_MID = 128
LX = L // N_CORES          # 512 query rows per core
N_MC = D_MODEL // 128      # 8 contraction chunks of 128
N_JT = L // 512            # 8 z-column tiles of 512
INV_SQRT_D = 1.0 / math.sqrt(D_ATTN)

F32 = mybir.dt.float32
F32R = mybir.dt.float32r
BF16 = mybir.dt.bfloat16
BF16_NP = ml_dtypes.bfloat16

# test.py sets these to get tracing / timing out of the same code path
TRACE = False
LAST_RESULT = None


def build():
    nc = bacc.Bacc("TRN2", target_bir_lowering=False)

    # Inputs, pre-packed on host so each DMA is one contiguous read:
    #  xc  [128p, 8c, 512i]       x-shard transposed+chunked (c = d_model chunk)
    #  zr  [8jt, 128p, 8c, 512j]  z transposed+chunked+tiled by j
    #  wpack [128p, wk|wv|wq|identb]  (each w as 8c x 128d)
    #  fpack [128p, bq|bk|bv | ones | ident]
    xc_e = nc.declare_dram_parameter("xc", [128, N_MC, LX], BF16, isOutput=False)
    zr_e = nc.declare_dram_parameter("zr", [N_JT, 128, N_MC, 512], BF16, isOutput=False)
    # wpack = wk | wv | wq | identb  along the free dim, all bf16
    wpack_e = nc.declare_dram_parameter(
        "wpack", [128, 3 * N_MC * 128 + 128], BF16, isOutput=False
    )
    # fpack = bs(3) | ones(1) | ident(128)  fp32
    fpack_e = nc.declare_dram_parameter("fpack", [128, 4 + 128], F32, isOutput=False)
    # out [128p, 4c, 128e]: y row i = c*128+p  (host re-interleaves)
    out_e = nc.declare_dram_parameter("out", [128, LX // 128, D_MID], F32, isOutput=True)


    with TileContext(nc) as tc:
        with (
            tc.tile_pool(name="consts", bufs=1) as consts,
            tc.tile_pool(name="zpool", bufs=8) as zpool,
            tc.tile_pool(name="kpool", bufs=3) as kpool,
            tc.tile_pool(name="vpool", bufs=3) as vpool,
            tc.tile_pool(name="vnpool", bufs=2) as vnpool,
            tc.tile_pool(name="ppool", bufs=6) as ppool,
            tc.tile_pool(name="epil", bufs=1) as epil,
            tc.tile_pool(name="ps_mm", bufs=3, space="PSUM") as ps_mm,
            tc.tile_pool(name="ps_s", bufs=3, space="PSUM") as ps_s,
            tc.tile_pool(name="ps_t", bufs=1, space="PSUM") as ps_t,
            tc.tile_pool(name="ps_acc", bufs=1, space="PSUM") as ps_acc,
        ):
            # ---- zt0 + k/v-path constants first so tile-0 matmuls start early
            zt0 = zpool.tile([128, N_MC, 512], BF16, name="zt0", tag="zt")
            nc.sync.dma_start(out=zt0[:, 0:4, :], in_=zr_e[0, :, 0:4, :])
            nc.sync.dma_start(out=zt0[:, 4:8, :], in_=zr_e[0, :, 4:8, :])

            wpk = consts.tile([128, 3 * N_MC * 128 + 128], BF16)
            W = N_MC * 128
            nc.scalar.dma_start(out=wpk[:, 0:W], in_=wpack_e[:, 0:W])
            nc.scalar.dma_start(out=wpk[:, W:], in_=wpack_e[:, W:])
            fpk = consts.tile([128, 4 + 128], F32)
            nc.scalar.dma_start(out=fpk, in_=fpack_e[:, :])
            wk_s = wpk[:, 0 * W:1 * W].rearrange("p (c d) -> p c d", c=N_MC)
            wv_s = wpk[:, 1 * W:2 * W].rearrange("p (c d) -> p c d", c=N_MC)
            wq_s = wpk[:, 2 * W:3 * W].rearrange("p (c d) -> p c d", c=N_MC)
            identb = wpk[:, 3 * W:3 * W + 128]
            bs_s = fpk[:, 0:3]
            ident = fpk[:, 4:132]

            # q-path input (behind zt0/wpack in the DMA queues)
            xc_s = consts.tile([128, N_MC, LX], BF16)
            nc.scalar.dma_start(out=xc_s, in_=xc_e[:, :, :])

            # persistent accumulators: yT [e, i] (PSUM) and the partition-wise
            # softmax-denominator partial sums (SBUF, DVE-accumulated)
            ps_y = ps_acc.tile([128, LX], F32, name="ps_y", tag="ps_y")
            rs_acc = consts.tile([128, LX], F32R)
            rs_acc2 = consts.tile([128, LX], F32)

            qT_s = consts.tile([128, LX], BF16)

            # rowsum-reduction constants, prepared during the prologue
            ones4f = epil.tile([128, 4], F32)
            nc.vector.memset(ones4f, 1.0)
            ones4 = epil.tile([128, 4], F32R)
            nc.vector.tensor_copy(ones4, ones4f)

            def kv_tile(jt, zt):
                """kT tile + v-natural tile for z-column tile jt."""
                ps_k = ps_mm.tile([128, 512], F32, name=f"ps_k{jt}", tag="mm")
                for c in range(N_MC):
                    nc.tensor.matmul(
                        ps_k, wk_s[:, c, :], zt[:, c, :],
                        start=(c == 0), stop=(c == N_MC - 1),
                    )
                kt = kpool.tile([128, 512], BF16, name=f"kt{jt}", tag="kt")
                nc.vector.tensor_scalar_add(kt, ps_k, bs_s[:, 1:2])

                ps_v = ps_mm.tile([128, 512], F32, name=f"ps_v{jt}", tag="mm")
                for c in range(N_MC):
                    nc.tensor.matmul(
                        ps_v, wv_s[:, c, :], zt[:, c, :],
                        start=(c == 0), stop=(c == N_MC - 1),
                    )
                vt = vpool.tile([128, 512], BF16, name=f"vt{jt}", tag="vt")
                nc.vector.tensor_scalar_add(vt, ps_v, bs_s[:, 2:3])
                # v natural [j, e] chunks via PE transpose (bf16, 1 cyc/row)
                vn = vnpool.tile([128, 4, D_MID], BF16, name=f"vn{jt}", tag="vn")
                ps_tv = ps_t.tile([128, 4, 128], BF16, name=f"ps_tv{jt}", tag="ps_t")
                for s4 in range(4):
                    nc.tensor.transpose(
                        ps_tv[:, s4, :], vt[:, s4 * 128:(s4 + 1) * 128], identb
                    )
                nc.vector.tensor_copy(vn, ps_tv)
                return kt, vn

            def attn_tile(jt, kt, vn):
                for s4 in range(4):
                    ps_st = ps_s.tile([128, LX], F32, name=f"ps_st{jt}_{s4}", tag="ps_st")
                    nc.tensor.matmul(
                        ps_st, kt[:, s4 * 128:(s4 + 1) * 128], qT_s,
                        start=True, stop=True,
                    )
                    pt = ppool.tile([128, LX], BF16, name=f"pt{jt}_{s4}", tag="pt")
                    nc.scalar.activation(
                        pt, ps_st, mybir.ActivationFunctionType.Exp, scale=INV_SQRT_D
                    )
                    first = jt == 0 and s4 == 0
                    last = jt == N_JT - 1 and s4 == 3
                    nc.tensor.matmul(ps_y, vn[:, s4, :], pt, start=first, stop=last)
                    # rowsum partials: even s4 chain on DVE, odd s4 on GpSimd
                    if jt == 0 and s4 < 2:
                        eng = nc.vector if s4 == 0 else nc.gpsimd
                        eng.tensor_copy(rs_acc if s4 == 0 else rs_acc2, pt)
                    elif s4 % 2 == 0:
                        nc.vector.tensor_add(rs_acc, rs_acc, pt)
                    else:
                        nc.gpsimd.tensor_add(rs_acc2, rs_acc2, pt)

            # kv stage runs one z-tile ahead of the attention stage so the
            # in-order PE never stalls on the q-path prologue DMAs
            zt1 = zpool.tile([128, N_MC, 512], BF16, name="zt1", tag="zt")
            nc.sync.dma_start(out=zt1, in_=zr_e[1, :, :, :])

            kv = [kv_tile(0, zt0), kv_tile(1, zt1)]

            # qT [d, i] = sum_c Wq[c].T @ xT[c]  (+bq)
            ps_q = ps_mm.tile([128, LX], F32, name="ps_q", tag="mm")
            for c in range(N_MC):
                nc.tensor.matmul(
                    ps_q, wq_s[:, c, :], xc_s[:, c, :],
                    start=(c == 0), stop=(c == N_MC - 1),
                )
            nc.scalar.activation(
                qT_s, ps_q, mybir.ActivationFunctionType.Identity, bias=bs_s[:, 0:1]
            )

            for jt in range(N_JT):
                if jt + 2 < N_JT:
                    zt = zpool.tile(
                        [128, N_MC, 512], BF16, name=f"zt{jt + 2}", tag="zt"
                    )
                    nc.sync.dma_start(out=zt, in_=zr_e[jt + 2, :, :, :])
                    kv.append(kv_tile(jt + 2, zt))
                kt, vn = kv[jt]
                attn_tile(jt, kt, vn)

            # ---- epilogue ----
            # Emit the yT leg first: its DVE copy only needs the last y-MM,
            # while the rowsum merge below must wait for the GpSimd chain —
            # keeping the copy ahead of the merge in DVE's in-order queue
            # lets the PE transposes start ~2us earlier.
            yT_s = epil.tile([128, LX], F32)
            nc.vector.tensor_copy(yT_s, ps_y)
            y_out = epil.tile([128, LX // 128, D_MID], F32)
            ps_yt = ps_t.tile([128, LX // 128, 128], F32, name="ps_yt", tag="ps_t")
            for c in range(LX // 128):
                nc.tensor.transpose(
                    ps_yt[:, c, :], yT_s[:, c * 128:(c + 1) * 128], ident
                )

            # rs_col[p, c] = sum_j rs_acc[j, c*128+p] directly via K=128
            # f32r matmuls (lhsT = rs_acc chunk, rhs = ones; N=4 because the
            # ISA rejects N=1 for f32r)
            nc.vector.tensor_add(rs_acc, rs_acc, rs_acc2)
            ps_rc = ps_s.tile([128, LX // 128, 4], F32, name="ps_rc", tag="ps_st")
            for c in range(LX // 128):
                nc.tensor.matmul(
                    ps_rc[:, c, :], rs_acc[:, c * 128:(c + 1) * 128], ones4,
                    start=True, stop=True,
                )
            rsr = epil.tile([128, LX // 128], F32)
            nc.vector.reciprocal(rsr, ps_rc[:, :, 0])

            for c in range(LX // 128):
                nc.vector.tensor_scalar_mul(y_out[:, c, :], ps_yt[:, c, :], rsr[:, c:c + 1])
                nc.sync.dma_start(out=out_e[:, c, :], in_=y_out[:, c, :])

    nc.compile()
    return nc


def _pack_kxm(w):
    """[D_MODEL, d] -> [128p, 8c*d] bf16 with m = c*128 + p."""
    d = w.shape[1]
    return (
        w.reshape(N_MC, 128, d).transpose(1, 0, 2).astype(BF16_NP).reshape(128, -1)
    )


def kernel(x, z, Wq, bq, Wk, bk, Wv, bv):
    global LAST_RESULT
    x = np.asarray(x, dtype=np.float32)
    z = np.asarray(z, dtype=np.float32)

    zT = np.ascontiguousarray(z.T)                      # [1024, 4096]
    # [8c, 128p, 8jt, 512j] -> [jt, p, c, j]
    zr = np.ascontiguousarray(
        zT.reshape(N_MC, 128, N_JT, 512).transpose(2, 1, 0, 3).astype(BF16_NP)
    )
    xT = np.ascontiguousarray(x.T)                      # [1024, 4096]
    bs = np.ascontiguousarray(
        np.stack(
            [
                np.asarray(bq, dtype=np.float32),
                np.asarray(bk, dtype=np.float32),
                np.asarray(bv, dtype=np.float32),
            ],
            axis=1,
        )
    )
    wpack = np.ascontiguousarray(
        np.concatenate(
            [
                _pack_kxm(np.asarray(Wk, dtype=np.float32)),
                _pack_kxm(np.asarray(Wv, dtype=np.float32)),
                _pack_kxm(np.asarray(Wq, dtype=np.float32)),
                np.eye(128, dtype=BF16_NP),
            ],
            axis=1,
        )
    )
    fpack = np.ascontiguousarray(
        np.concatenate(
            [bs, np.ones((128, 1), np.float32), np.eye(128, dtype=np.float32)],
            axis=1,
        )
    )

    in_maps = []
    for cid in range(N_CORES):
        xs = xT[:, cid * LX:(cid + 1) * LX]             # [1024, 512]
        xc = np.ascontiguousarray(
            xs.reshape(N_MC, 128, LX).transpose(1, 0, 2).astype(BF16_NP)
        )
        in_maps.append({"xc": xc, "zr": zr, "wpack": wpack, "fpack": fpack})

    nc = build()
    res = run_bass_kernel_spmd(
        nc, in_maps, core_ids=list(range(N_CORES)), trace=TRACE
    )
    LAST_RESULT = res

    out = np.empty((L, D_MID), dtype=np.float32)
    for cid in range(N_CORES):
        o = res.results[cid]["out"]                     # [128, 4, 128]
        out[cid * LX:(cid + 1) * LX] = np.asarray(o).transpose(1, 0, 2).reshape(LX, D_MID)
    return out


# revision 35
# speedup vs baseline: 1.0078x; 1.0078x over previous
"""Distributed cross-attention kernel for 8 TRN2 NeuronCores.

Reference computation (L=4096, D_MODEL=1024, D_ATTN=D_MID=128):
    q = x @ Wq + bq ; k = z @ Wk + bk ; v = z @ Wv + bv
    y = softmax(q @ k.T / sqrt(128)) @ v

Sharding: query rows (L_x) split 8 ways; each core holds its x shard and a
replicated copy of z / weights, computes k/v locally, and runs a
flash-attention-style pipeline over 8 z-column tiles of 512.

Matmul inputs are bf16 (accumulation stays fp32 in PSUM; softmax sums and
the normalization run in fp32), which halves HBM traffic and runs the PE
at full rate with pipelined weight loads.  No max-subtraction in the
softmax: s ~ N(0,1) here so exp() is safely bounded.

Host-side work is layout only: transpose/pack/cast the inputs into the
exact SBUF tile layouts (so every DMA is one contiguous read), and
re-stack the 8 output shards.
"""
import math
import sys

import numpy as np

sys.path.insert(0, "/opt/trn_rl_repo")

import ml_dtypes  # noqa: E402

import concourse.mybir as mybir  # noqa: E402
from concourse import bacc  # noqa: E402
from concourse.bass_utils import run_bass_kernel_spmd  # noqa: E402
from concourse.tile import TileContext  # noqa: E402

N_CORES = 8
L = 4096
D_MODEL = 1024
D_ATTN = 128
D_MID = 128
LX = L // N_CORES          # 512 query rows per core
N_MC = D_MODEL // 128      # 8 contraction chunks of 128
N_JT = L // 512            # 8 z-column tiles of 512
INV_SQRT_D = 1.0 / math.sqrt(D_ATTN)

F32 = mybir.dt.float32
F32R = mybir.dt.float32r
BF16 = mybir.dt.bfloat16
BF16_NP = ml_dtypes.bfloat16

# test.py sets these to get tracing / timing out of the same code path
TRACE = False
LAST_RESULT = None


def build():
    nc = bacc.Bacc("TRN2", target_bir_lowering=False)

    # Inputs, pre-packed on host so each DMA is one contiguous read:
    #  xc  [128p, 8c, 512i]       x-shard transposed+chunked (c = d_model chunk)
    #  zr  [8jt, 128p, 8c, 512j]  z transposed+chunked+tiled by j
    #  wpack [128p, wk|wv|wq|identb]  (each w as 8c x 128d)
    #  fpack [128p, bq|bk|bv | ones | ident]
    xc_e = nc.declare_dram_parameter("xc", [128, N_MC, LX], BF16, isOutput=False)
    zr_e = nc.declare_dram_parameter("zr", [N_JT, 128, N_MC, 512], BF16, isOutput=False)
    # wpack = wk | wv | wq | identb  along the free dim, all bf16
    wpack_e = nc.declare_dram_parameter(
        "wpack", [128, 3 * N_MC * 128 + 128], BF16, isOutput=False
    )
    # fpack = bs(3) | ones(1) | ident(128)  fp32
    fpack_e = nc.declare_dram_parameter("fpack", [128, 4 + 128], F32, isOutput=False)
    # out [128p, 4c, 128e]: y row i = c*128+p  (host re-interleaves)
    out_e = nc.declare_dram_parameter("out", [128, LX // 128, D_MID], F32, isOutput=True)


    with TileContext(nc) as tc:
        with (
            tc.tile_pool(name="consts", bufs=1) as consts,
            tc.tile_pool(name="zpool", bufs=8) as zpool,
            tc.tile_pool(name="kpool", bufs=3) as kpool,
            tc.tile_pool(name="vpool", bufs=3) as vpool,
            tc.tile_pool(name="vnpool", bufs=2) as vnpool,
            tc.tile_pool(name="ppool", bufs=6) as ppool,
            tc.tile_pool(name="epil", bufs=1) as epil,
            tc.tile_pool(name="ps_mm", bufs=3, space="PSUM") as ps_mm,
            tc.tile_pool(name="ps_s", bufs=3, space="PSUM") as ps_s,
            tc.tile_pool(name="ps_t", bufs=1, space="PSUM") as ps_t,
            tc.tile_pool(name="ps_acc", bufs=1, space="PSUM") as ps_acc,
        ):
            # ---- zt0 + k/v-path constants first so tile-0 matmuls start early
            zt0 = zpool.tile([128, N_MC, 512], BF16, name="zt0", tag="zt")
            nc.sync.dma_start(out=zt0[:, 0:4, :], in_=zr_e[0, :, 0:4, :])
            nc.sync.dma_start(out=zt0[:, 4:8, :], in_=zr_e[0, :, 4:8, :])

            wpk = consts.tile([128, 3 * N_MC * 128 + 128], BF16)
            W = N_MC * 128
            nc.scalar.dma_start(out=wpk[:, 0:W], in_=wpack_e[:, 0:W])
            nc.scalar.dma_start(out=wpk[:, W:], in_=wpack_e[:, W:])
            fpk = consts.tile([128, 4 + 128], F32)
            nc.scalar.dma_start(out=fpk, in_=fpack_e[:, :])
            wk_s = wpk[:, 0 * W:1 * W].rearrange("p (c d) -> p c d", c=N_MC)
            wv_s = wpk[:, 1 * W:2 * W].rearrange("p (c d) -> p c d", c=N_MC)
            wq_s = wpk[:, 2 * W:3 * W].rearrange("p (c d) -> p c d", c=N_MC)
            identb = wpk[:, 3 * W:3 * W + 128]
            bs_s = fpk[:, 0:3]
            ident = fpk[:, 4:132]

            # q-path input (behind zt0/wpack in the DMA queues)
            xc_s = consts.tile([128, N_MC, LX], BF16)
            nc.scalar.dma_start(out=xc_s, in_=xc_e[:, :, :])

            # persistent accumulators: yT [e, i] (PSUM) and the partition-wise
            # softmax-denominator partial sums (SBUF, DVE-accumulated)
            ps_y = ps_acc.tile([128, LX], F32, name="ps_y", tag="ps_y")
            rs_acc = consts.tile([128, LX], F32R)
            rs_acc2 = consts.tile([128, LX], F32)

            qT_s = consts.tile([128, LX], BF16)

            # rowsum-reduction constants, prepared during the prologue
            ones4f = epil.tile([128, 4], F32)
            nc.vector.memset(ones4f, 1.0)
            ones4 = epil.tile([128, 4], F32R)
            nc.vector.tensor_copy(ones4, ones4f)

            def kv_tile(jt, zt):
                """kT tile + v-natural tile for z-column tile jt."""
                ps_k = ps_mm.tile([128, 512], F32, name=f"ps_k{jt}", tag="mm")
                for c in range(N_MC):
                    nc.tensor.matmul(
                        ps_k, wk_s[:, c, :], zt[:, c, :],
                        start=(c == 0), stop=(c == N_MC - 1),
                    )
                kt = kpool.tile([128, 512], BF16, name=f"kt{jt}", tag="kt")
                nc.vector.tensor_scalar_add(kt, ps_k, bs_s[:, 1:2])

                ps_v = ps_mm.tile([128, 512], F32, name=f"ps_v{jt}", tag="mm")
                for c in range(N_MC):
                    nc.tensor.matmul(
                        ps_v, wv_s[:, c, :], zt[:, c, :],
                        start=(c == 0), stop=(c == N_MC - 1),
                    )
                vt = vpool.tile([128, 512], BF16, name=f"vt{jt}", tag="vt")
                nc.vector.tensor_scalar_add(vt, ps_v, bs_s[:, 2:3])
                # v natural [j, e] chunks via PE transpose (bf16, 1 cyc/row)
                vn = vnpool.tile([128, 4, D_MID], BF16, name=f"vn{jt}", tag="vn")
                ps_tv = ps_t.tile([128, 4, 128], BF16, name=f"ps_tv{jt}", tag="ps_t")
                for s4 in range(4):
                    nc.tensor.transpose(
                        ps_tv[:, s4, :], vt[:, s4 * 128:(s4 + 1) * 128], identb
                    )
                nc.vector.tensor_copy(vn, ps_tv)
                return kt, vn

            def attn_tile(jt, kt, vn):
                for s4 in range(4):
                    ps_st = ps_s.tile([128, LX], F32, name=f"ps_st{jt}_{s4}", tag="ps_st")
                    nc.tensor.matmul(
                        ps_st, kt[:, s4 * 128:(s4 + 1) * 128], qT_s,
                        start=True, stop=True,
                    )
                    pt = ppool.tile([128, LX], BF16, name=f"pt{jt}_{s4}", tag="pt")
                    nc.scalar.activation(
                        pt, ps_st, mybir.ActivationFunctionType.Exp, scale=INV_SQRT_D
                    )
                    first = jt == 0 and s4 == 0
                    last = jt == N_JT - 1 and s4 == 3
                    nc.tensor.matmul(ps_y, vn[:, s4, :], pt, start=first, stop=last)
                    # rowsum partials: even s4 chain on DVE, odd s4 on GpSimd
                    if jt == 0 and s4 < 2:
                        eng = nc.vector if s4 == 0 else nc.gpsimd
                        eng.tensor_copy(rs_acc if s4 == 0 else rs_acc2, pt)
                    elif s4 % 2 == 0 or jt == N_JT - 1:
                        nc.vector.tensor_add(rs_acc, rs_acc, pt)
                    else:
                        nc.gpsimd.tensor_add(rs_acc2, rs_acc2, pt)

            # kv stage runs one z-tile ahead of the attention stage so the
            # in-order PE never stalls on the q-path prologue DMAs
            zt1 = zpool.tile([128, N_MC, 512], BF16, name="zt1", tag="zt")
            nc.sync.dma_start(out=zt1, in_=zr_e[1, :, :, :])

            kv = [kv_tile(0, zt0), kv_tile(1, zt1)]

            # qT [d, i] = sum_c Wq[c].T @ xT[c]  (+bq)
            ps_q = ps_mm.tile([128, LX], F32, name="ps_q", tag="mm")
            for c in range(N_MC):
                nc.tensor.matmul(
                    ps_q, wq_s[:, c, :], xc_s[:, c, :],
                    start=(c == 0), stop=(c == N_MC - 1),
                )
            nc.scalar.activation(
                qT_s, ps_q, mybir.ActivationFunctionType.Identity, bias=bs_s[:, 0:1]
            )

            for jt in range(N_JT):
                if jt + 2 < N_JT:
                    zt = zpool.tile(
                        [128, N_MC, 512], BF16, name=f"zt{jt + 2}", tag="zt"
                    )
                    nc.sync.dma_start(out=zt, in_=zr_e[jt + 2, :, :, :])
                    kv.append(kv_tile(jt + 2, zt))
                kt, vn = kv[jt]
                attn_tile(jt, kt, vn)

            # ---- epilogue ----
            # Emit the yT leg first: its DVE copy only needs the last y-MM,
            # while the rowsum merge below must wait for the GpSimd chain —
            # keeping the copy ahead of the merge in DVE's in-order queue
            # lets the PE transposes start ~2us earlier.
            yT_s = epil.tile([128, LX], F32)
            nc.scalar.activation(
                yT_s, ps_y, mybir.ActivationFunctionType.Identity, bias=0.0
            )
            y_out = epil.tile([128, LX // 128, D_MID], F32)
            ps_yt = ps_t.tile([128, LX // 128, 128], F32, name="ps_yt", tag="ps_t")
            for c in range(LX // 128):
                nc.tensor.transpose(
                    ps_yt[:, c, :], yT_s[:, c * 128:(c + 1) * 128], ident
                )

            # rs_col[p, c] = sum_j rs_acc[j, c*128+p] directly via K=128
            # f32r matmuls (lhsT = rs_acc chunk, rhs = ones; N=4 because the
            # ISA rejects N=1 for f32r)
            nc.vector.tensor_add(rs_acc, rs_acc, rs_acc2)
            ps_rc = ps_s.tile([128, LX // 128, 4], F32, name="ps_rc", tag="ps_st")
            for c in range(LX // 128):
                nc.tensor.matmul(
                    ps_rc[:, c, :], rs_acc[:, c * 128:(c + 1) * 128], ones4,
                    start=True, stop=True,
                )
            rsr = epil.tile([128, LX // 128], F32)
            nc.vector.reciprocal(rsr, ps_rc[:, :, 0])

            for c in range(LX // 128):
                nc.vector.tensor_scalar_mul(y_out[:, c, :], ps_yt[:, c, :], rsr[:, c:c + 1])
                nc.sync.dma_start(out=out_e[:, c, :], in_=y_out[:, c, :])

    nc.compile()
    return nc


def _pack_kxm(w):
    """[D_MODEL, d] -> [128p, 8c*d] bf16 with m = c*128 + p."""
    d = w.shape[1]
    return (
        w.reshape(N_MC, 128, d).transpose(1, 0, 2).astype(BF16_NP).reshape(128, -1)
    )


def kernel(x, z, Wq, bq, Wk, bk, Wv, bv):
    global LAST_RESULT
    x = np.asarray(x, dtype=np.float32)
    z = np.asarray(z, dtype=np.float32)

    zT = np.ascontiguousarray(z.T)                      # [1024, 4096]
    # [8c, 128p, 8jt, 512j] -> [jt, p, c, j]
    zr = np.ascontiguousarray(
        zT.reshape(N_MC, 128, N_JT, 512).transpose(2, 1, 0, 3).astype(BF16_NP)
    )
    xT = np.ascontiguousarray(x.T)                      # [1024, 4096]
    bs = np.ascontiguousarray(
        np.stack(
            [
                np.asarray(bq, dtype=np.float32),
                np.asarray(bk, dtype=np.float32),
                np.asarray(bv, dtype=np.float32),
            ],
            axis=1,
        )
    )
    wpack = np.ascontiguousarray(
        np.concatenate(
            [
                _pack_kxm(np.asarray(Wk, dtype=np.float32)),
                _pack_kxm(np.asarray(Wv, dtype=np.float32)),
                _pack_kxm(np.asarray(Wq, dtype=np.float32)),
                np.eye(128, dtype=BF16_NP),
            ],
            axis=1,
        )
    )
    fpack = np.ascontiguousarray(
        np.concatenate(
            [bs, np.ones((128, 1), np.float32), np.eye(128, dtype=np.float32)],
            axis=1,
        )
    )

    in_maps = []
    for cid in range(N_CORES):
        xs = xT[:, cid * LX:(cid + 1) * LX]             # [1024, 512]
        xc = np.ascontiguousarray(
            xs.reshape(N_MC, 128, LX).transpose(1, 0, 2).astype(BF16_NP)
        )
        in_maps.append({"xc": xc, "zr": zr, "wpack": wpack, "fpack": fpack})

    nc = build()
    res = run_bass_kernel_spmd(
        nc, in_maps, core_ids=list(range(N_CORES)), trace=TRACE
    )
    LAST_RESULT = res

    out = np.empty((L, D_MID), dtype=np.float32)
    for cid in range(N_CORES):
        o = res.results[cid]["out"]                     # [128, 4, 128]
        out[cid * LX:(cid + 1) * LX] = np.asarray(o).transpose(1, 0, 2).reshape(LX, D_MID)
    return out


# revision 36
# speedup vs baseline: 1.0097x; 1.0019x over previous
"""Distributed cross-attention kernel for 8 TRN2 NeuronCores.

Reference computation (L=4096, D_MODEL=1024, D_ATTN=D_MID=128):
    q = x @ Wq + bq ; k = z @ Wk + bk ; v = z @ Wv + bv
    y = softmax(q @ k.T / sqrt(128)) @ v

Sharding: query rows (L_x) split 8 ways; each core holds its x shard and a
replicated copy of z / weights, computes k/v locally, and runs a
flash-attention-style pipeline over 8 z-column tiles of 512.

Matmul inputs are bf16 (accumulation stays fp32 in PSUM; softmax sums and
the normalization run in fp32), which halves HBM traffic and runs the PE
at full rate with pipelined weight loads.  No max-subtraction in the
softmax: s ~ N(0,1) here so exp() is safely bounded.

Host-side work is layout only: transpose/pack/cast the inputs into the
exact SBUF tile layouts (so every DMA is one contiguous read), and
re-stack the 8 output shards.
"""
import math
import sys

import numpy as np

sys.path.insert(0, "/opt/trn_rl_repo")

import ml_dtypes  # noqa: E402

import concourse.mybir as mybir  # noqa: E402
from concourse import bacc  # noqa: E402
from concourse.bass_utils import run_bass_kernel_spmd  # noqa: E402
from concourse.tile import TileContext  # noqa: E402

N_CORES = 8
L = 4096
D_MODEL = 1024
D_ATTN = 128
D_MID = 128
LX = L // N_CORES          # 512 query rows per core
N_MC = D_MODEL // 128      # 8 contraction chunks of 128
N_JT = L // 512            # 8 z-column tiles of 512
INV_SQRT_D = 1.0 / math.sqrt(D_ATTN)

F32 = mybir.dt.float32
F32R = mybir.dt.float32r
BF16 = mybir.dt.bfloat16
BF16_NP = ml_dtypes.bfloat16

# test.py sets these to get tracing / timing out of the same code path
TRACE = False
LAST_RESULT = None


def build():
    nc = bacc.Bacc("TRN2", target_bir_lowering=False)

    # Inputs, pre-packed on host so each DMA is one contiguous read:
    #  xc  [128p, 8c, 512i]       x-shard transposed+chunked (c = d_model chunk)
    #  zr  [8jt, 128p, 8c, 512j]  z transposed+chunked+tiled by j
    #  wpack [128p, wk|wv|wq|identb]  (each w as 8c x 128d)
    #  fpack [128p, bq|bk|bv | ones | ident]
    xc_e = nc.declare_dram_parameter("xc", [128, N_MC, LX], BF16, isOutput=False)
    zr_e = nc.declare_dram_parameter("zr", [N_JT, 128, N_MC, 512], BF16, isOutput=False)
    # wpack = wk | wv | wq | identb  along the free dim, all bf16
    wpack_e = nc.declare_dram_parameter(
        "wpack", [128, 3 * N_MC * 128 + 128], BF16, isOutput=False
    )
    # fpack = bs(3) | ones(1) | ident(128)  fp32
    fpack_e = nc.declare_dram_parameter("fpack", [128, 4 + 128], F32, isOutput=False)
    # out [128p, 4c, 128e]: y row i = c*128+p  (host re-interleaves)
    out_e = nc.declare_dram_parameter("out", [128, LX // 128, D_MID], F32, isOutput=True)


    with TileContext(nc) as tc:
        with (
            tc.tile_pool(name="consts", bufs=1) as consts,
            tc.tile_pool(name="zpool", bufs=8) as zpool,
            tc.tile_pool(name="kpool", bufs=3) as kpool,
            tc.tile_pool(name="vpool", bufs=3) as vpool,
            tc.tile_pool(name="vnpool", bufs=2) as vnpool,
            tc.tile_pool(name="ppool", bufs=6) as ppool,
            tc.tile_pool(name="epil", bufs=1) as epil,
            tc.tile_pool(name="ps_mm", bufs=3, space="PSUM") as ps_mm,
            tc.tile_pool(name="ps_s", bufs=3, space="PSUM") as ps_s,
            tc.tile_pool(name="ps_t", bufs=1, space="PSUM") as ps_t,
            tc.tile_pool(name="ps_acc", bufs=1, space="PSUM") as ps_acc,
        ):
            # ---- zt0 + k/v-path constants first so tile-0 matmuls start early
            zt0 = zpool.tile([128, N_MC, 512], BF16, name="zt0", tag="zt")
            nc.sync.dma_start(out=zt0[:, 0:4, :], in_=zr_e[0, :, 0:4, :])
            nc.sync.dma_start(out=zt0[:, 4:8, :], in_=zr_e[0, :, 4:8, :])

            wpk = consts.tile([128, 3 * N_MC * 128 + 128], BF16)
            W = N_MC * 128
            nc.scalar.dma_start(out=wpk[:, 0:W], in_=wpack_e[:, 0:W])
            nc.scalar.dma_start(out=wpk[:, W:], in_=wpack_e[:, W:])
            fpk = consts.tile([128, 4 + 128], F32)
            nc.scalar.dma_start(out=fpk, in_=fpack_e[:, :])
            wk_s = wpk[:, 0 * W:1 * W].rearrange("p (c d) -> p c d", c=N_MC)
            wv_s = wpk[:, 1 * W:2 * W].rearrange("p (c d) -> p c d", c=N_MC)
            wq_s = wpk[:, 2 * W:3 * W].rearrange("p (c d) -> p c d", c=N_MC)
            identb = wpk[:, 3 * W:3 * W + 128]
            bs_s = fpk[:, 0:3]
            ident = fpk[:, 4:132]

            # q-path input (behind zt0/wpack in the DMA queues)
            xc_s = consts.tile([128, N_MC, LX], BF16)
            nc.scalar.dma_start(out=xc_s, in_=xc_e[:, :, :])

            # persistent accumulators: yT [e, i] (PSUM) and the partition-wise
            # softmax-denominator partial sums (SBUF, DVE-accumulated)
            ps_y = ps_acc.tile([128, LX], F32, name="ps_y", tag="ps_y")
            rs_acc = consts.tile([128, LX], F32R)
            rs_acc2 = consts.tile([128, LX], F32)

            qT_s = consts.tile([128, LX], BF16)

            # rowsum-reduction constants, prepared during the prologue
            ones4f = epil.tile([128, 4], F32)
            nc.vector.memset(ones4f, 1.0)
            ones4 = epil.tile([128, 4], F32R)
            nc.vector.tensor_copy(ones4, ones4f)

            def kv_tile(jt, zt):
                """kT tile + v-natural tile for z-column tile jt.

                For tile 0 the two DMA halves arrive staggered, so k and v
                accumulation interleave by half: the v matmuls on half 1
                fill the PE while half 2 is still streaming in.
                """
                ps_k = ps_mm.tile([128, 512], F32, name=f"ps_k{jt}", tag="mm")
                ps_v = ps_mm.tile([128, 512], F32, name=f"ps_v{jt}", tag="mm")
                order = (
                    [("k", c) for c in range(4)] + [("v", c) for c in range(4)]
                    + [("k", c) for c in range(4, 8)] + [("v", c) for c in range(4, 8)]
                ) if jt == 0 else (
                    [("k", c) for c in range(N_MC)] + [("v", c) for c in range(N_MC)]
                )
                for which, c in order:
                    ps, w_s = (ps_k, wk_s) if which == "k" else (ps_v, wv_s)
                    nc.tensor.matmul(
                        ps, w_s[:, c, :], zt[:, c, :],
                        start=(c == 0), stop=(c == N_MC - 1),
                    )
                kt = kpool.tile([128, 512], BF16, name=f"kt{jt}", tag="kt")
                nc.vector.tensor_scalar_add(kt, ps_k, bs_s[:, 1:2])
                vt = vpool.tile([128, 512], BF16, name=f"vt{jt}", tag="vt")
                nc.vector.tensor_scalar_add(vt, ps_v, bs_s[:, 2:3])
                # v natural [j, e] chunks via PE transpose (bf16, 1 cyc/row)
                vn = vnpool.tile([128, 4, D_MID], BF16, name=f"vn{jt}", tag="vn")
                ps_tv = ps_t.tile([128, 4, 128], BF16, name=f"ps_tv{jt}", tag="ps_t")
                for s4 in range(4):
                    nc.tensor.transpose(
                        ps_tv[:, s4, :], vt[:, s4 * 128:(s4 + 1) * 128], identb
                    )
                nc.vector.tensor_copy(vn, ps_tv)
                return kt, vn

            def attn_tile(jt, kt, vn):
                for s4 in range(4):
                    ps_st = ps_s.tile([128, LX], F32, name=f"ps_st{jt}_{s4}", tag="ps_st")
                    nc.tensor.matmul(
                        ps_st, kt[:, s4 * 128:(s4 + 1) * 128], qT_s,
                        start=True, stop=True,
                    )
                    pt = ppool.tile([128, LX], BF16, name=f"pt{jt}_{s4}", tag="pt")
                    nc.scalar.activation(
                        pt, ps_st, mybir.ActivationFunctionType.Exp, scale=INV_SQRT_D
                    )
                    first = jt == 0 and s4 == 0
                    last = jt == N_JT - 1 and s4 == 3
                    nc.tensor.matmul(ps_y, vn[:, s4, :], pt, start=first, stop=last)
                    # rowsum partials: even s4 chain on DVE, odd s4 on GpSimd
                    if jt == 0 and s4 < 2:
                        eng = nc.vector if s4 == 0 else nc.gpsimd
                        eng.tensor_copy(rs_acc if s4 == 0 else rs_acc2, pt)
                    elif s4 % 2 == 0 or jt == N_JT - 1:
                        nc.vector.tensor_add(rs_acc, rs_acc, pt)
                    else:
                        nc.gpsimd.tensor_add(rs_acc2, rs_acc2, pt)

            # kv stage runs one z-tile ahead of the attention stage so the
            # in-order PE never stalls on the q-path prologue DMAs
            zt1 = zpool.tile([128, N_MC, 512], BF16, name="zt1", tag="zt")
            nc.sync.dma_start(out=zt1, in_=zr_e[1, :, :, :])

            kv = [kv_tile(0, zt0), kv_tile(1, zt1)]

            # qT [d, i] = sum_c Wq[c].T @ xT[c]  (+bq)
            ps_q = ps_mm.tile([128, LX], F32, name="ps_q", tag="mm")
            for c in range(N_MC):
                nc.tensor.matmul(
                    ps_q, wq_s[:, c, :], xc_s[:, c, :],
                    start=(c == 0), stop=(c == N_MC - 1),
                )
            nc.scalar.activation(
                qT_s, ps_q, mybir.ActivationFunctionType.Identity, bias=bs_s[:, 0:1]
            )

            for jt in range(N_JT):
                if jt + 2 < N_JT:
                    zt = zpool.tile(
                        [128, N_MC, 512], BF16, name=f"zt{jt + 2}", tag="zt"
                    )
                    nc.sync.dma_start(out=zt, in_=zr_e[jt + 2, :, :, :])
                    kv.append(kv_tile(jt + 2, zt))
                kt, vn = kv[jt]
                attn_tile(jt, kt, vn)

            # ---- epilogue ----
            # Emit the yT leg first: its DVE copy only needs the last y-MM,
            # while the rowsum merge below must wait for the GpSimd chain —
            # keeping the copy ahead of the merge in DVE's in-order queue
            # lets the PE transposes start ~2us earlier.
            yT_s = epil.tile([128, LX], F32)
            nc.scalar.activation(
                yT_s, ps_y, mybir.ActivationFunctionType.Identity, bias=0.0
            )
            y_out = epil.tile([128, LX // 128, D_MID], F32)
            ps_yt = ps_t.tile([128, LX // 128, 128], F32, name="ps_yt", tag="ps_t")
            for c in range(LX // 128):
                nc.tensor.transpose(
                    ps_yt[:, c, :], yT_s[:, c * 128:(c + 1) * 128], ident
                )

            # rs_col[p, c] = sum_j rs_acc[j, c*128+p] directly via K=128
            # f32r matmuls (lhsT = rs_acc chunk, rhs = ones; N=4 because the
            # ISA rejects N=1 for f32r)
            nc.vector.tensor_add(rs_acc, rs_acc, rs_acc2)
            ps_rc = ps_s.tile([128, LX // 128, 4], F32, name="ps_rc", tag="ps_st")
            for c in range(LX // 128):
                nc.tensor.matmul(
                    ps_rc[:, c, :], rs_acc[:, c * 128:(c + 1) * 128], ones4,
                    start=True, stop=True,
                )
            rsr = epil.tile([128, LX // 128], F32)
            nc.vector.reciprocal(rsr, ps_rc[:, :, 0])

            for c in range(LX // 128):
                nc.vector.tensor_scalar_mul(y_out[:, c, :], ps_yt[:, c, :], rsr[:, c:c + 1])
                nc.sync.dma_start(out=out_e[:, c, :], in_=y_out[:, c, :])

    nc.compile()
    return nc


def _pack_kxm(w):
    """[D_MODEL, d] -> [128p, 8c*d] bf16 with m = c*128 + p."""
    d = w.shape[1]
    return (
        w.reshape(N_MC, 128, d).transpose(1, 0, 2).astype(BF16_NP).reshape(128, -1)
    )


def kernel(x, z, Wq, bq, Wk, bk, Wv, bv):
    global LAST_RESULT
    x = np.asarray(x, dtype=np.float32)
    z = np.asarray(z, dtype=np.float32)

    zT = np.ascontiguousarray(z.T)                      # [1024, 4096]
    # [8c, 128p, 8jt, 512j] -> [jt, p, c, j]
    zr = np.ascontiguousarray(
        zT.reshape(N_MC, 128, N_JT, 512).transpose(2, 1, 0, 3).astype(BF16_NP)
    )
    xT = np.ascontiguousarray(x.T)                      # [1024, 4096]
    bs = np.ascontiguousarray(
        np.stack(
            [
                np.asarray(bq, dtype=np.float32),
                np.asarray(bk, dtype=np.float32),
                np.asarray(bv, dtype=np.float32),
            ],
            axis=1,
        )
    )
    wpack = np.ascontiguousarray(
        np.concatenate(
            [
                _pack_kxm(np.asarray(Wk, dtype=np.float32)),
                _pack_kxm(np.asarray(Wv, dtype=np.float32)),
                _pack_kxm(np.asarray(Wq, dtype=np.float32)),
                np.eye(128, dtype=BF16_NP),
            ],
            axis=1,
        )
    )
    fpack = np.ascontiguousarray(
        np.concatenate(
            [bs, np.ones((128, 1), np.float32), np.eye(128, dtype=np.float32)],
            axis=1,
        )
    )

    in_maps = []
    for cid in range(N_CORES):
        xs = xT[:, cid * LX:(cid + 1) * LX]             # [1024, 512]
        xc = np.ascontiguousarray(
            xs.reshape(N_MC, 128, LX).transpose(1, 0, 2).astype(BF16_NP)
        )
        in_maps.append({"xc": xc, "zr": zr, "wpack": wpack, "fpack": fpack})

    nc = build()
    res = run_bass_kernel_spmd(
        nc, in_maps, core_ids=list(range(N_CORES)), trace=TRACE
    )
    LAST_RESULT = res

    out = np.empty((L, D_MID), dtype=np.float32)
    for cid in range(N_CORES):
        o = res.results[cid]["out"]                     # [128, 4, 128]
        out[cid * LX:(cid + 1) * LX] = np.asarray(o).transpose(1, 0, 2).reshape(LX, D_MID)
    return out
